# revision 1
# baseline (speedup 1.0000x reference)
"""CapsuleLayer dynamic-routing kernel for 8x TRN2 NeuronCores.

Sharding: C=32 capsules split 4-per-core (routing is fully independent per
capsule). Each core computes priors = einsum('bni,cnio->cbno') for its 4
capsules via 512 PE matmuls (one per pair of route nodes n), keeps priors
resident in SBUF, and runs the 3 routing iterations locally.

Matmul mapping (per pair j, n0=2j, n1=2j+1):
  stationary lhsT = rw[(dn,i)=128, (c,o)=128]   (fp32, fused weight load)
  moving     rhs  = x block-diag [(dn,i)=128, (b,dn')=32]
  out[(c,o), (b,dn')] = priors[c, b, 2j+dn', o]
Priors land in PSUM with (c,o) on partitions -> evacuated to P_T[co=128,
(b,n)=16K] (b-major) which serves both routing reductions:
  delta[c,b,n] = sum_o P.v : PE matmul, lhsT = c-block-diag(v) [128,4]
  s[c,b,o] = sum_n p.P     : PE broadcast of p over o + DVE fused
                             multiply-reduce (scalar_tensor_tensor accum_out)
Everything fp32 (routing softmax is argmax-like; bf16/tf32 priors flip
routing decisions - measured 2-6% final error).
"""

import os
import sys
import numpy as np

sys.path.insert(0, "/opt/trn_rl_repo")

import concourse.bass as bass
import concourse.bacc as bacc
import concourse.mybir as mybir
from concourse.tile import TileContext
from concourse.bass_utils import run_bass_kernel_spmd

F32 = mybir.dt.float32
F32R = mybir.dt.float32r
AF = mybir.ActivationFunctionType
ALU = mybir.AluOpType
USE_F32R = os.environ.get("CAPS_F32R", "0") == "1"


def _mmcast(ap):
    return ap.bitcast(F32R) if USE_F32R else ap

B, N, CI = 16, 1024, 64
C, O = 32, 32
NCORES = 8
CLOC = C // NCORES          # 4 capsules per core
NP2 = N // 2                # 512 pairs
G = 32                      # pair groups (16 pairs each)
PG = NP2 // G               # 16 pairs per group

LAST_RESULTS = None


def build_bass(stop_after=None):
    nc = bacc.Bacc(None, target_bir_lowering=False)

    rwx_h = nc.dram_tensor("rwx", [G, 128, PG * 144], F32, kind="ExternalInput")
    # ones4[c', c*32+o] = (c==c')
    ones4_h = nc.dram_tensor("ones4", [CLOC, 128], F32, kind="ExternalInput")
    # ones_km[(c,o), c'] = (c==c')
    oneskm_h = nc.dram_tensor("ones_km", [128, CLOC], F32, kind="ExternalInput")
    out_h = nc.dram_tensor("out", [128, B], F32, kind="ExternalOutput")

    with TileContext(nc) as tc:
        with (
            tc.tile_pool(name="persist", bufs=1) as persist,
            tc.tile_pool(name="consts", bufs=1) as consts,
            tc.tile_pool(name="dramp", bufs=1, space="DRAM") as dramp,
            tc.tile_pool(name="stg", bufs=3) as stg,
        ):
            # Persistent SBUF
            PT = persist.tile([128, B * N], F32, tag="PT")          # priors, b-major
            acc = persist.tile([128, 512], F32, tag="acc")          # s1 accumulator
            Lg = dramp.tile([CLOC, B * N], F32, tag="logits")       # logits [4,(b,n)] DRAM
            D2 = dramp.tile([CLOC, B * N], F32, tag="delta2")       # delta2 [4,(b,n)] DRAM
            p4d = dramp.tile([CLOC, B * N], F32, tag="p4d")         # probs DRAM bounce
            p4 = persist.tile([CLOC, B * N], F32, tag="p4")         # probs  [4,(b,n)]
            sm64 = persist.tile([64, N], F32, tag="sm64")           # softmax work
            e64 = persist.tile([64, N], F32, tag="e64")
            vcur = persist.tile([128, B], F32, tag="vcur")          # squash out
            scur = persist.tile([128, B], F32, tag="scur")          # s input
            Vbd = persist.tile([128, 4 * B], F32, tag="Vbd")        # blockdiag v
            stt_dummy = persist.tile([128, 512], F32, tag="sttd")   # small scratch
            stt_dummy2 = persist.tile([128, N], F32, tag="sttd2")   # STT main out

            ones4 = consts.tile([CLOC, 128], F32, tag="ones4")
            oneskm = consts.tile([128, CLOC], F32, tag="oneskm")
            nc.sync.dma_start(out=ones4[:, :], in_=ones4_h[:, :])
            nc.sync.dma_start(out=oneskm[:, :], in_=oneskm_h[:, :])

            nc.gpsimd.memset(acc[:, :], 0.0)

            # ---------------- main phase: priors ----------------
            # block-diag x staging ring: 3 slots of [128, 512], zeros persist
            xz = persist.tile([128, 4 * 512 + 16], F32, tag="xz")
            nc.gpsimd.memset(xz[:, :], 0.0)
            with (
                tc.tile_pool(name="rwp", bufs=4) as rwp,
                tc.tile_pool(name="mps", bufs=6, space="PSUM") as mps,
            ):
                for g in range(G):
                    rwt = rwp.tile([128, PG * 144], F32, tag="rw")
                    nc.sync.dma_start(out=rwt[:, :], in_=rwx_h[g])
                    xoff = PG * 128
                    zoff = (g % 4) * 512
                    # scatter dense x halves into block-diag slots
                    for d in range(2):
                        dstv = xz[64 * d:64 * (d + 1),
                                  zoff + 16 * d:zoff + 16 * d + 512] \
                            .rearrange("k (p b) -> k p b", b=2 * B)[:, :, :B]
                        nc.vector.tensor_copy(
                            dstv,
                            rwt[64 * d:64 * (d + 1), xoff:xoff + 256]
                            .rearrange("k (p b) -> k p b", b=B))
                    ps = mps.tile([128, 512], F32, tag="mm")
                    for p in range(PG):
                        nc.tensor.matmul(
                            ps[:, 32 * p:32 * (p + 1)],
                            rwt[:, 128 * p:128 * (p + 1)],
                            xz[:, zoff + 32 * p:zoff + 32 * (p + 1)],
                            start=True, stop=True,
                        )
                    # evac: psum free=(p,dn',b) idx 32p+16dn'+b -> PT (b-major)
                    src = ps[:, :].rearrange("k (p d b) -> k p b d", p=PG, b=B, d=2)
                    dst = PT[:, :].rearrange("k (b n) -> k b n", b=B) \
                                  [:, :, 32 * g:32 * (g + 1)] \
                                  .rearrange("k b (p d) -> k p b d", p=PG, d=2)
                    nc.scalar.copy(out=dst, in_=src)
                    # s1 accumulation from evacuated chunk (keeps PSUM readers=1)
                    ptv = PT[:, :].rearrange("k (b n) -> k b n", b=B)[
                        :, :, 32 * g:32 * (g + 1)]
                    nc.vector.tensor_add(acc[:, :], acc[:, :], ptv)

            # s1 = sum over n-sub of acc, keep b
            accv = acc[:, :].rearrange("k (b n) -> k b n", b=B)
            nc.vector.tensor_reduce(
                scur[:, :], accv, axis=mybir.AxisListType.X, op=ALU.add)

            def squash(pre_scale):
                # scur [128(co), B] -> vcur [128(co), B]
                with tc.tile_pool(name="sqp", bufs=2, space="PSUM") as sqp:
                    if pre_scale != 1.0:
                        nc.vector.tensor_scalar_mul(scur[:, :], scur[:, :], pre_scale)
                    s2t = stt_dummy[:, :B]
                    nc.vector.tensor_mul(s2t, scur[:, :], scur[:, :])
                    sq_ps = sqp.tile([CLOC, B], F32, tag="sq")
                    nc.tensor.matmul(sq_ps[:, :], oneskm[:, :], s2t,
                                     start=True, stop=True)
                    lnt = sm64[:CLOC, :B]
                    rt = sm64[:CLOC, B:2 * B]
                    dt_ = sm64[:CLOC, 2 * B:3 * B]
                    sct = sm64[:CLOC, 3 * B:4 * B]
                    # r = sqrt(sq) = exp(0.5*ln(sq))
                    nc.scalar.activation(lnt, sq_ps[:, :], AF.Ln)
                    nc.scalar.activation(rt, lnt, AF.Exp, scale=0.5)
                    nc.vector.tensor_scalar_add(dt_, sq_ps[:, :], 1.0)
                    nc.vector.reciprocal(dt_, dt_)
                    nc.vector.tensor_mul(sct, rt, dt_)
                    screp = sqp.tile([128, B], F32, tag="screp")
                    nc.tensor.matmul(screp[:, :], ones4[:, :], sct,
                                     start=True, stop=True)
                    nc.vector.tensor_mul(vcur[:, :], scur[:, :], screp[:, :])

            def build_vbd():
                nc.gpsimd.memset(Vbd[:, :], 0.0)
                vb = Vbd[:, :].rearrange("k (b c) -> k b c", b=B, c=CLOC)
                for cp in range(CLOC):
                    nc.vector.tensor_copy(
                        vb[32 * cp:32 * (cp + 1), :, cp],
                        vcur[32 * cp:32 * (cp + 1), :])

            def delta_pass(first):
                # delta[c',b,n] = sum_o P.v ; logits += delta (or =)
                with tc.tile_pool(name="dps", bufs=2, space="PSUM") as dps:
                    for b in range(B):
                        dp = dps.tile([CLOC, N], F32, tag="d")
                        for h in range(2):
                            nc.tensor.matmul(
                                dp[:, 512 * h:512 * (h + 1)],
                                _mmcast(Vbd[:, 4 * b:4 * (b + 1)]),
                                _mmcast(PT[:, N * b + 512 * h:N * b + 512 * (h + 1)]),
                                start=True, stop=True)
                        st = stg.tile([CLOC, N], F32, tag="st")
                        nc.vector.tensor_copy(st[:, :], dp[:, :])
                        tgt = Lg if first else D2
                        nc.sync.dma_start(out=tgt[:, N * b:N * (b + 1)],
                                          in_=st[:, :])

            def softmax(add_d2=False):
                # Lg(dram) [4,(b,n)] -> sm64 [64(b,c),n] -> p4 [4,(b,n)]
                nc.sync.dma_start(
                    out=sm64[:, :],
                    in_=Lg[:, :].rearrange("c (b n) -> b c n", b=B))
                if add_d2:
                    tmp = stt_dummy2[:64, :N]
                    nc.sync.dma_start(
                        out=tmp,
                        in_=D2[:, :].rearrange("c (b n) -> b c n", b=B))
                    nc.vector.tensor_add(sm64[:, :], sm64[:, :], tmp)
                mx = stt_dummy[:64, :1]
                nc.vector.tensor_reduce(mx, sm64[:, :],
                                        axis=mybir.AxisListType.X, op=ALU.max)
                nc.vector.tensor_scalar_mul(mx, mx, -1.0)
                nc.scalar.activation(e64[:, :], sm64[:, :], AF.Exp, bias=mx)
                sume = stt_dummy[:64, 1:2]
                nc.vector.tensor_reduce(sume, e64[:, :],
                                        axis=mybir.AxisListType.X, op=ALU.add)
                nc.vector.reciprocal(sume, sume)
                nc.vector.tensor_scalar_mul(e64[:, :], e64[:, :], sume)
                nc.sync.dma_start(
                    out=p4d[:, :].rearrange("c (b n) -> b c n", b=B),
                    in_=e64[:, :])
                nc.sync.dma_start(out=p4[:, :], in_=p4d[:, :])

            def s_pass():
                # s[c,b,o] = sum_n p[c,b,n] * P_T[(c,o),(b,n)]
                with tc.tile_pool(name="prp", bufs=3, space="PSUM") as prp:
                    for b in range(B):
                        pr = prp.tile([128, N], F32, tag="prep")
                        for h in range(2):
                            nc.tensor.matmul(
                                pr[:, 512 * h:512 * (h + 1)], _mmcast(ones4[:, :]),
                                _mmcast(p4[:, N * b + 512 * h:N * b + 512 * (h + 1)]),
                                start=True, stop=True)
                        nc.vector.scalar_tensor_tensor(
                            out=stt_dummy2[:, :],
                            in0=PT[:, N * b:N * (b + 1)],
                            scalar=1.0,
                            in1=pr[:, :],
                            op0=ALU.mult, op1=ALU.mult,
                            accum_out=scur[:, b:b + 1])

            # ---------------- routing ----------------
            steps = [
                ("sq1", lambda: (squash(1.0 / N), build_vbd())),
                ("d1", lambda: delta_pass(first=True)),
                ("sm2", softmax),
                ("s2", s_pass),
                ("sq2", lambda: (squash(1.0), build_vbd())),
                ("d2", lambda: delta_pass(first=False)),
                ("sm3", lambda: softmax(add_d2=True)),
                ("s3", s_pass),
                ("sq3", lambda: squash(1.0)),
            ]
            for name, fn in steps:
                fn()
                if stop_after == name:
                    break
            nc.sync.dma_start(out=out_h[:, :], in_=vcur[:, :])

    return nc


def shard_inputs(x, rw):
    """Build per-core input maps (host-side layout marshaling only)."""
    x = np.ascontiguousarray(x, dtype=np.float32)
    rw = np.ascontiguousarray(rw, dtype=np.float32)
    # dense xc[g,(dn,i),(p,b)] = x[b, 32g+2p+dn, i]
    xg = x.reshape(B, G, PG, 2, CI)              # b,g,p,dn,i
    xg = xg.transpose(1, 3, 4, 2, 0)             # g,dn,i,p,b
    xc = np.ascontiguousarray(xg).reshape(G, 128, PG * B)  # [G,128,256]

    ones4 = np.zeros((CLOC, 128), dtype=np.float32)
    oneskm = np.zeros((128, CLOC), dtype=np.float32)
    for cp in range(CLOC):
        ones4[cp, 32 * cp:32 * (cp + 1)] = 1.0
        oneskm[32 * cp:32 * (cp + 1), cp] = 1.0

    in_maps = []
    for k in range(NCORES):
        rwk = rw[CLOC * k:CLOC * (k + 1)]        # c,n,i,o
        rwg = rwk.reshape(CLOC, G, PG, 2, CI, O)  # c,g,p,dn,i,o
        rwg = rwg.transpose(1, 3, 4, 2, 0, 5)     # g,dn,i,p,c,o
        rwg = np.ascontiguousarray(rwg).reshape(G, 128, PG * 128)
        rwx = np.concatenate([rwg, xc], axis=2)   # [G,128,2304]
        in_maps.append({
            "rwx": rwx,
            "ones4": ones4,
            "ones_km": oneskm,
        })
    return in_maps


def kernel(x, route_weights):
    global LAST_RESULTS
    in_dtype = x.dtype
    nc = build_bass()
    if not nc.is_finalized():
        nc.finalize()
    in_maps = shard_inputs(np.asarray(x), np.asarray(route_weights))
    os.environ["BASS_NEVER_TRACE"] = "1"  # axon ntff hook missing in this env
    res = run_bass_kernel_spmd(
        nc, in_maps, core_ids=list(range(NCORES)), trace=False,
    )
    LAST_RESULTS = res
    out = np.zeros((C, B, 1, 1, O), dtype=np.float32)
    for k in range(NCORES):
        o_np = res.results[k]["out"]             # [128, B]
        o_np = o_np.reshape(CLOC, O, B)          # c,o,b
        out[CLOC * k:CLOC * (k + 1), :, 0, 0, :] = o_np.transpose(0, 2, 1)
    return out.astype(in_dtype, copy=False)


if __name__ == "__main__":
    import reference
    inputs = {k: np.asarray(v) for k, v in reference.setup_inputs().items()}
    got = kernel(**inputs)
    print("kernel output shape:", got.shape)



# revision 6
# speedup vs baseline: 3.5088x; 3.5088x over previous
"""CapsuleLayer dynamic-routing kernel for 8x TRN2 NeuronCores — v2.

Sharding: C=32 capsules split 4-per-core. Each core computes priors for its 4
capsules and runs the 3 routing iterations locally.

v2 changes vs baseline:
- fp16 route_weights + x for the priors matmuls (PSUM accumulates fp32):
  halves the dominant HBM traffic (37.7 -> 18.9 MB/core) and enables FWL
  (2x faster weight loads; fp32 gets no FWL and 4 cyc/row).
- Routing restructured to eliminate all DRAM bounces:
  * delta: 16 accumulating fp16 matmuls with per-batch block-diag stationary
    V_b -> logits land directly in PSUM as [64=(b,c), N] (softmax layout).
  * probs broadcast over o via constant fp16 selection matmuls reading the
    softmax output [64,(n)] directly (no [4,(b,n)] transpose round-trip).
  * softmax: fused exp+sum via ScalarE activation accum_out.
- Priors stay fp32 in SBUF (PT) for the s-contraction (DVE STT); a second
  fp16 copy (PT16) feeds the delta matmuls. Routing decisions tolerate the
  ~5e-4 fp16 rounding (bf16's 4e-3 does flip them; fp16 measured safe).
"""

import os
import sys
import numpy as np

sys.path.insert(0, "/opt/trn_rl_repo")

import concourse.bass as bass
import concourse.bacc as bacc
import concourse.mybir as mybir
from concourse.tile import TileContext
from concourse.bass_utils import run_bass_kernel_spmd

F32 = mybir.dt.float32
F16 = mybir.dt.float16
AF = mybir.ActivationFunctionType
ALU = mybir.AluOpType

B, N, CI = 16, 1024, 64
C, O = 32, 32
NCORES = 8
CLOC = C // NCORES          # 4 capsules per core
NP2 = N // 2                # 512 pairs
G = 8                       # pair groups (64 pairs each -> 2.36 MB DMAs)
PG = NP2 // G               # 32 pairs per group
SLOT = PG * 32              # xz block-diag slot width

LAST_RESULTS = None


def build_bass(reps=1, stop_after=None, routing_only=False):
    nc = bacc.Bacc(None, target_bir_lowering=False)

    rwx_h = nc.dram_tensor("rwx", [G, 128, PG * 144], F16, kind="ExternalInput")
    sel_h = nc.dram_tensor("sel", [64, B * 128], F16, kind="ExternalInput")
    ones4_h = nc.dram_tensor("ones4", [CLOC, 128], F32, kind="ExternalInput")
    oneskm_h = nc.dram_tensor("ones_km", [128, CLOC], F32, kind="ExternalInput")
    out_h = nc.dram_tensor("out", [128, B], F32, kind="ExternalOutput")

    with TileContext(nc) as tc:
        with (
            tc.tile_pool(name="persist", bufs=1) as persist,
            tc.tile_pool(name="consts", bufs=1) as consts,
        ):
            PT = persist.tile([128, B * N], F32, tag="PT")      # priors, b-major
            PT16 = persist.tile([128, B * N], F16, tag="PT16")  # fp16 copy
            acc = persist.tile([128, B * 2 * PG], F32, tag="acc")  # s1 accum
            e16 = persist.tile([64, N], F16, tag="e16")         # probs fp16
            VbdT = persist.tile([128, 1088], F16, tag="VbdT")   # 16 stationary V_b
            scur = persist.tile([128, B], F32, tag="scur")
            vcur = persist.tile([128, B], F32, tag="vcur")
            sm4 = persist.tile([CLOC, 4 * B], F32, tag="sm4")   # squash scratch
            red = persist.tile([64, 4], F32, tag="red")         # mx/sume/rec
            sdum = persist.tile([128, N], F32, tag="sdum")      # STT main out

            ones4 = consts.tile([CLOC, 128], F32, tag="ones4")
            oneskm = consts.tile([128, CLOC], F32, tag="oneskm")
            sel = consts.tile([64, B * 128], F16, tag="sel")
            nc.sync.dma_start(out=ones4[:, :], in_=ones4_h[:, :])
            nc.sync.dma_start(out=oneskm[:, :], in_=oneskm_h[:, :])
            nc.sync.dma_start(out=sel[:, :], in_=sel_h[:, :])

            nc.gpsimd.memset(VbdT[:, :], 0.0)

            # ---------------- priors (fp16 matmuls, fp32 PSUM) ----------------
            xz = persist.tile([128, 4 * SLOT + 16], F16, tag="xz")
            nc.gpsimd.memset(xz[:, :], 0.0)

            def priors_phase():
              nc.gpsimd.memset(acc[:, :], 0.0)
              with (
                tc.tile_pool(name="rwp", bufs=3) as rwp,
                tc.tile_pool(name="mps", bufs=2, space="PSUM") as mps,
              ):
                for g in range(G):
                    rwt = rwp.tile([128, PG * 144], F16, tag="rw")
                    nc.sync.dma_start(out=rwt[:, :], in_=rwx_h[g])
                    xoff = PG * 128
                    zoff = (g % 4) * SLOT
                    for d in range(2):
                        dstv = xz[64 * d:64 * (d + 1),
                                  zoff + 16 * d:zoff + 16 * d + SLOT] \
                            .rearrange("k (p b) -> k p b", b=2 * B)[:, :, :B]
                        nc.vector.tensor_copy(
                            dstv,
                            rwt[64 * d:64 * (d + 1), xoff:xoff + PG * B]
                            .rearrange("k (p b) -> k p b", b=B))
                    ps = mps.tile([128, SLOT], F32, tag="mm")
                    for p in range(PG):
                        nc.tensor.matmul(
                            ps[:, 32 * p:32 * (p + 1)],
                            rwt[:, 128 * p:128 * (p + 1)],
                            xz[:, zoff + 32 * p:zoff + 32 * (p + 1)],
                            start=True, stop=True,
                        )
                    # evac: psum free=(p,dn',b) -> PT/PT16 (b-major)
                    npg = 2 * PG    # nodes per group
                    src = ps[:, :].rearrange("k (p d b) -> k p b d", p=PG, b=B, d=2)
                    dstF = PT[:, :].rearrange("k (b n) -> k b n", b=B) \
                                   [:, :, npg * g:npg * (g + 1)] \
                                   .rearrange("k b (p d) -> k p b d", p=PG, d=2)
                    dstH = PT16[:, :].rearrange("k (b n) -> k b n", b=B) \
                                     [:, :, npg * g:npg * (g + 1)] \
                                     .rearrange("k b (p d) -> k p b d", p=PG, d=2)
                    nc.scalar.copy(out=dstF, in_=src)
                    nc.vector.tensor_copy(dstH, src)
                    ptv = PT[:, :].rearrange("k (b n) -> k b n", b=B)[
                        :, :, npg * g:npg * (g + 1)]
                    nc.vector.tensor_add(acc[:, :], acc[:, :], ptv)

              # s1 = sum over n of priors (uniform probs), keep b
              accv = acc[:, :].rearrange("k (b n) -> k b n", b=B)
              nc.vector.tensor_reduce(
                  scur[:, :], accv, axis=mybir.AxisListType.X, op=ALU.add)

            def squash(pre_scale):
                # scur [128(co), B] -> vcur [128(co), B]
                with tc.tile_pool(name="sqp", bufs=1, space="PSUM") as sqp:
                    if pre_scale != 1.0:
                        nc.vector.tensor_scalar_mul(scur[:, :], scur[:, :],
                                                    pre_scale)
                    s2t = sdum[:, :B]
                    nc.vector.tensor_mul(s2t, scur[:, :], scur[:, :])
                    sq_ps = sqp.tile([CLOC, B], F32, tag="sq")
                    nc.tensor.matmul(sq_ps[:, :], oneskm[:, :], s2t,
                                     start=True, stop=True)
                    rt = sm4[:, B:2 * B]
                    dt_ = sm4[:, 2 * B:3 * B]
                    sct = sm4[:, 3 * B:4 * B]
                    # scale = sqrt(sq)/(1+sq)
                    nc.scalar.activation(rt, sq_ps[:, :], AF.Sqrt)
                    nc.vector.tensor_scalar_add(dt_, sq_ps[:, :], 1.0)
                    nc.vector.reciprocal(dt_, dt_)
                    nc.vector.tensor_mul(sct, rt, dt_)
                    screp = sqp.tile([128, B], F32, tag="screp")
                    nc.tensor.matmul(screp[:, :], ones4[:, :], sct,
                                     start=True, stop=True)
                    nc.vector.tensor_mul(vcur[:, :], scur[:, :], screp[:, :])

            def build_vbd():
                # V_b stationary tiles VbdT[:, 64b:64b+64]; nonzero col of
                # tile b for capsule cp sits at global col 68b+cp.
                vb = VbdT[:, :].rearrange("k (b r) -> k b r", b=B, r=68)
                for cp in range(CLOC):
                    nc.vector.tensor_copy(
                        vb[32 * cp:32 * (cp + 1), :, cp:cp + 1],
                        vcur[32 * cp:32 * (cp + 1), :]
                        .rearrange("k (b u) -> k b u", u=1))

            def delta_pass(dpsA, dpsB, first):
                # logits-delta[(b,c), n] = sum_o P16.v via 16 accumulating
                # matmuls (only rows b'==b of matmul b are nonzero). The
                # second pass accumulates straight onto the first pass's
                # PSUM, so logits live in PSUM across the whole routing.
                for b in range(B):
                    for h, dps in ((0, dpsA), (1, dpsB)):
                        nc.tensor.matmul(
                            dps[:, :],
                            VbdT[:, 64 * b:64 * b + 64],
                            PT16[:, N * b + 512 * h:N * b + 512 * (h + 1)],
                            start=(first and b == 0), stop=(b == B - 1),
                            skip_group_check=not first,
                        )

            def softmax(dpsA, dpsB):
                mxA = red[:, 0:1]
                mxB = red[:, 1:2]
                nc.vector.tensor_reduce(mxA, dpsA[:, :],
                                        axis=mybir.AxisListType.X, op=ALU.max)
                nc.vector.tensor_reduce(mxB, dpsB[:, :],
                                        axis=mybir.AxisListType.X, op=ALU.max)
                nc.vector.tensor_max(mxA, mxA, mxB)
                nc.vector.tensor_scalar_mul(mxA, mxA, -1.0)
                sA = red[:, 2:3]
                sB = red[:, 3:4]
                nc.scalar.activation(e16[:, :512], dpsA[:, :], AF.Exp,
                                     bias=mxA, accum_out=sA)
                nc.scalar.activation(e16[:, 512:], dpsB[:, :], AF.Exp,
                                     bias=mxA, accum_out=sB)
                nc.vector.tensor_add(sA, sA, sB)
                nc.vector.reciprocal(sA, sA)
                nc.vector.tensor_scalar_mul(e16[:, :], e16[:, :], sA)

            def s_pass():
                # s[c,b,o] = sum_n p[c,b,n] * PT[(c,o),(b,n)]
                with tc.tile_pool(name="prp", bufs=2, space="PSUM") as prp:
                    for b in range(B):
                        pr = prp.tile([128, N], F32, tag="prep")
                        for h in range(2):
                            nc.tensor.matmul(
                                pr[:, 512 * h:512 * (h + 1)],
                                sel[:, 128 * b:128 * (b + 1)],
                                e16[:, 512 * h:512 * (h + 1)],
                                start=True, stop=True)
                        nc.vector.scalar_tensor_tensor(
                            out=sdum[:, :],
                            in0=PT[:, N * b:N * (b + 1)],
                            scalar=1.0,
                            in1=pr[:, :],
                            op0=ALU.mult, op1=ALU.mult,
                            accum_out=scur[:, b:b + 1])

            # ---------------- routing ----------------
            if routing_only:
                priors_phase()
            for _rep in range(reps):
                pre = ([] if routing_only else [("priors", priors_phase)]) + [
                    ("sq1", lambda: (squash(1.0 / N), build_vbd())),
                ]
                stopped = False
                for name, fn in pre:
                    fn()
                    if stop_after == name:
                        stopped = True
                        break
                if not stopped:
                    with tc.tile_pool(name="dps", bufs=1,
                                      space="PSUM") as dpsp:
                        dpsA = dpsp.tile([64, 512], F32, tag="dA")
                        dpsB = dpsp.tile([64, 512], F32, tag="dB")
                        steps = [
                            ("d1", lambda: delta_pass(dpsA, dpsB, first=True)),
                            ("sm2", lambda: softmax(dpsA, dpsB)),
                            ("s2", s_pass),
                            ("sq2", lambda: (squash(1.0), build_vbd())),
                            ("d2", lambda: delta_pass(dpsA, dpsB,
                                                      first=False)),
                            ("sm3", lambda: softmax(dpsA, dpsB)),
                            ("s3", s_pass),
                            ("sq3", lambda: squash(1.0)),
                        ]
                        for name, fn in steps:
                            fn()
                            if stop_after == name:
                                break
                src = vcur if stop_after is None else scur
                nc.sync.dma_start(out=out_h[:, :], in_=src[:, :])

    return nc


def shard_inputs(x, rw):
    """Build per-core input maps (host-side layout marshaling only)."""
    x = np.ascontiguousarray(x, dtype=np.float32)
    rw = np.ascontiguousarray(rw, dtype=np.float32)
    # dense xc[g,(dn,i),(p,b)] = x[b, 32g+2p+dn, i]
    xg = x.reshape(B, G, PG, 2, CI)              # b,g,p,dn,i
    xg = xg.transpose(1, 3, 4, 2, 0)             # g,dn,i,p,b
    xc = np.ascontiguousarray(xg, dtype=np.float16).reshape(G, 128, PG * B)

    ones4 = np.zeros((CLOC, 128), dtype=np.float32)
    oneskm = np.zeros((128, CLOC), dtype=np.float32)
    for cp in range(CLOC):
        ones4[cp, 32 * cp:32 * (cp + 1)] = 1.0
        oneskm[32 * cp:32 * (cp + 1), cp] = 1.0

    # sel[(b',c'), 128b + (c,o)] = (b'==b)&(c'==c)
    sel = np.zeros((64, B * 128), dtype=np.float16)
    for b in range(B):
        for cp in range(CLOC):
            sel[4 * b + cp, 128 * b + 32 * cp:128 * b + 32 * (cp + 1)] = 1.0

    in_maps = []
    for k in range(NCORES):
        rwk = rw[CLOC * k:CLOC * (k + 1)]        # c,n,i,o
        rwg = rwk.reshape(CLOC, G, PG, 2, CI, O)  # c,g,p,dn,i,o
        rwg = rwg.transpose(1, 3, 4, 2, 0, 5)     # g,dn,i,p,c,o
        rwg = np.ascontiguousarray(rwg, dtype=np.float16) \
                .reshape(G, 128, PG * 128)
        rwx = np.concatenate([rwg, xc], axis=2)   # [G,128,2304] fp16
        in_maps.append({
            "rwx": rwx,
            "sel": sel,
            "ones4": ones4,
            "ones_km": oneskm,
        })
    return in_maps


def kernel(x, route_weights):
    global LAST_RESULTS
    in_dtype = x.dtype
    nc = build_bass()
    if not nc.is_finalized():
        nc.finalize()
    in_maps = shard_inputs(np.asarray(x), np.asarray(route_weights))
    os.environ["BASS_NEVER_TRACE"] = "1"  # axon ntff hook missing in this env
    res = run_bass_kernel_spmd(
        nc, in_maps, core_ids=list(range(NCORES)), trace=False,
    )
    LAST_RESULTS = res
    out = np.zeros((C, B, 1, 1, O), dtype=np.float32)
    for k in range(NCORES):
        o_np = res.results[k]["out"]             # [128, B]
        o_np = o_np.reshape(CLOC, O, B)          # c,o,b
        out[CLOC * k:CLOC * (k + 1), :, 0, 0, :] = o_np.transpose(0, 2, 1)
    return out.astype(in_dtype, copy=False)


if __name__ == "__main__":
    import reference
    inputs = {k: np.asarray(v) for k, v in reference.setup_inputs().items()}
    got = kernel(**inputs)
    print("kernel output shape:", got.shape)
